# revision 7
# baseline (speedup 1.0000x reference)
"""Trainium2 Bass kernel: 3x3 VALID conv2d, stride 1.

Full input [32, 64, 112, 112] f32 + weights [128, 64, 3, 3] f32
-> output [32, 128, 110, 110] f32.

Data-parallel across 8 NeuronCores: 4 images per core.

Per-core formulation: conv as PE matmuls, out = lhsT.T @ rhs with
K (contraction, partitions) = 128 = two (ky,kx) taps x 64 channels,
M (out partitions) = 128 output channels,
N (moving free dim) = 4 input-width rows = 448 (<= 512, one PSUM bank).
The 2 rightmost columns of each 112-wide row are conv garbage; the
PSUM->SBUF copy compacts to the valid 110 columns.

Tap fusion (5 matmuls per chunk instead of the naive 9):
  tile1 = host-duplicated: partitions 0..63 = image rows h (shift (0,0)),
          partitions 64..127 = rows h+1 (shift (1,0)).
          m=0..2 fuse taps (0,kx)+(1,kx), rhs offset y0*W+kx.
  tile2 = host-duplicated second layout: partitions 0..63 = shift (2,0),
          partitions 64..127 = shift (2,1).
          m=3 fuses taps (2,0)+(2,1) at offset y0*W; m=4 is the lone
          (2,2) tap from half A at offset y0*W+2 (half-B weights zero).

Inputs are cast to fp16 on the host (full-rate PE streaming, fp32 PSUM
accumulation). Output is written fp16 (halves the store traffic) and
upcast to fp32 on the host; overall rel err ~5e-4.

Schedule: chunk-major in interleaved pairs (m0c0 m0c1 m1c0 .. m4c1) so
consecutive matmuls alternate PSUM banks and each chunk's PSUM drains
right after its stop matmul (weight reloads are per-matmul anyway and
hide under the 190ns stream). Copies split rows across vector+scalar.
All input DMAs ride the gpsimd SWDGE flood in consumption order
(weights first, tile1/tile2 bands interleaved, image 0 fine-grained);
outputs go on the SP HWDGE queue, one DMA per 4 chunks.
"""

import numpy as np

B_FULL = 32
N_CORES = 8
B_CORE = B_FULL // N_CORES  # 4 images per core
C_IN = 64
C_OUT = 128
H = W = 112
OH = OW = 110
XCOLS = 12322  # max col accessed: 108*112 + 2 + 224

_NC = None


def _build():
    from contextlib import ExitStack

    import concourse.tile as tile
    from concourse import bacc, mybir

    nc = bacc.Bacc("TRN2", target_bir_lowering=False, debug=False)
    x = nc.dram_tensor(
        "x", [B_CORE, 128, XCOLS], mybir.dt.float16, kind="ExternalInput"
    )
    x2 = nc.dram_tensor(
        "x2", [B_CORE, 128, XCOLS], mybir.dt.float16, kind="ExternalInput"
    )
    w = nc.dram_tensor("w", [128, 5, 128], mybir.dt.float16, kind="ExternalInput")
    y = nc.dram_tensor(
        "y", [B_CORE, C_OUT, OH, OW], mybir.dt.float16, kind="ExternalOutput"
    )

    chunks = []
    for b in range(B_CORE):
        y0 = 0
        for r in [4] * 27 + [2]:  # 27*4 + 2 = 110 output rows
            chunks.append((b, y0, r))
            y0 += r
    assert len(chunks) % 4 == 0

    # row bands, finer for image 0 (PE ramp); same grid for tile1/tile2
    BANDS0 = [0, 6, 16, 34, 61, 87, 111]
    BANDS = [0, 34, 61, 87, 111]

    with tile.TileContext(nc) as tc, ExitStack() as ctx:
        xpool = ctx.enter_context(tc.tile_pool(name="xp", bufs=4))
        x2pool = ctx.enter_context(tc.tile_pool(name="x2p", bufs=3))
        wpool = ctx.enter_context(tc.tile_pool(name="wp", bufs=1))
        opool = ctx.enter_context(tc.tile_pool(name="op", bufs=8))
        ppool = ctx.enter_context(tc.tile_pool(name="pp", bufs=8, space="PSUM"))

        wt = wpool.tile([128, 5, 128], mybir.dt.float16)
        nc.gpsimd.dma_start(wt[:], w.ap())

        xa = x.ap()
        x2a = x2.ap()
        ya = y.ap()

        xtiles = [None] * B_CORE
        x2tiles = [None] * B_CORE

        def issue_image_loads(b, t2_too=True):
            bands = BANDS0 if b == 0 else BANDS
            xt = xpool.tile([128, XCOLS], mybir.dt.float16, tag="xt")
            xtiles[b] = xt
            if t2_too:
                issue_x2_load(b)
            for lo, hi in zip(bands, bands[1:]):
                l, h2 = lo * W, min(hi * W, XCOLS)
                nc.gpsimd.dma_start(xt[:, l:h2], xa[b][:, l:h2])
                if t2_too:
                    nc.gpsimd.dma_start(
                        x2tiles[b][:, l:h2], x2a[b][:, l:h2]
                    )

        def issue_x2_load(b, bands=None):
            x2t = x2pool.tile([128, XCOLS], mybir.dt.float16, tag="x2t")
            x2tiles[b] = x2t
            if bands is not None:
                for lo, hi in zip(bands, bands[1:]):
                    l, h2 = lo * W, min(hi * W, XCOLS)
                    nc.gpsimd.dma_start(x2t[:, l:h2], x2a[b][:, l:h2])

        def issue_chunk_pair(c):
            """Two chunks, matmuls interleaved to alternate PSUM banks."""
            pair = chunks[c : c + 2]
            pts = []
            for b, y0, rows in pair:
                pts.append(
                    ppool.tile([128, 448], mybir.dt.float32, name="pt", tag="pt")
                )
            for m in range(5):
                for (b, y0, rows), pt in zip(pair, pts):
                    n = rows * W
                    if m < 3:
                        rhs = xtiles[b][:, y0 * W + m : y0 * W + m + n]
                    elif m == 3:
                        rhs = x2tiles[b][:, y0 * W : y0 * W + n]
                    else:
                        rhs = x2tiles[b][:, y0 * W + 2 : y0 * W + 2 + n]
                    nc.tensor.matmul(
                        pt[:, 0:n],
                        wt[:, m, :],
                        rhs,
                        start=(m == 0),
                        stop=(m == 4),
                        skip_group_check=True,
                    )
            return pair, pts

        def issue_quads(c0, c1):
            """One output tile + sync DMA per 4 chunks; per-chunk copies
            split rows across vector+scalar."""
            for c in range(c0, c1, 4):
                qchunks = chunks[c : c + 4]
                b0, y00, _ = qchunks[0]
                total_rows = sum(r for _, _, r in qchunks)
                assert all(b == b0 for b, _, _ in qchunks)
                ot = opool.tile([128, 16 * OW], mybir.dt.float16, tag="ot")
                off = 0
                for cc in (c, c + 2):
                    pair, pts = issue_chunk_pair(cc)
                    for (b, y0, rows), pt in zip(pair, pts):
                        psrc = pt[:].rearrange("p (r c) -> p r c", c=W)
                        odst = ot[:, off : off + rows * OW].rearrange(
                            "p (r c) -> p r c", c=OW
                        )
                        rh = rows // 2
                        nc.vector.tensor_copy(
                            odst[:, 0:rh], psrc[:, 0:rh, 0:OW]
                        )
                        nc.scalar.copy(
                            odst[:, rh:rows], psrc[:, rh:rows, 0:OW]
                        )
                        off += rows * OW
                nc.sync.dma_start(
                    ya[b0].rearrange("c h w -> c (h w)")[
                        :, y00 * OW : y00 * OW + total_rows * OW
                    ],
                    ot[:, 0 : total_rows * OW],
                )

        # issue order: x2 pool has 3 buffers; allocate img3's x2 third
        # (fresh buffer, loads early with no WAR wait -- it computes
        # last but its data must not arrive late), and img2's x2 fourth
        # (WAR on x2[0], released when image 0's chunks finish, still
        # well before image 2 computes at ~64us).
        issue_image_loads(0)
        issue_image_loads(1)
        issue_image_loads(3)
        issue_image_loads(2, t2_too=False)
        issue_quads(0, 28)  # image 0
        issue_x2_load(2, bands=BANDS)
        issue_quads(28, len(chunks))

    nc.compile()
    return nc


def _get_nc():
    global _NC
    if _NC is None:
        _NC = _build()
    return _NC


def _prep_weights(weights: np.ndarray) -> np.ndarray:
    # m=0..2: w5[ci, m, co] = w[co, ci, 0, m];  w5[64+ci, m, co] = w[co, ci, 1, m]
    # m=3:    w5[ci, 3, co] = w[co, ci, 2, 0];  w5[64+ci, 3, co] = w[co, ci, 2, 1]
    # m=4:    w5[ci, 4, co] = w[co, ci, 2, 2];  w5[64+ci, 4, co] = 0
    w = np.asarray(weights, dtype=np.float32)
    wt = w.transpose(1, 2, 3, 0)  # [ci, ky, kx, co]
    w5 = np.zeros((128, 5, 128), np.float32)
    w5[0:64, 0:3, :] = wt[:, 0, :, :]
    w5[64:128, 0:3, :] = wt[:, 1, :, :]
    w5[0:64, 3, :] = wt[:, 2, 0, :]
    w5[64:128, 3, :] = wt[:, 2, 1, :]
    w5[0:64, 4, :] = wt[:, 2, 2, :]
    return w5.astype(np.float16)


def kernel(input_image: np.ndarray, weights: np.ndarray, _trace: bool = False):
    from concourse.bass_utils import run_bass_kernel_spmd

    nc = _get_nc()
    x16 = np.asarray(input_image).astype(np.float16)  # [32, 64, 112, 112]
    # tile1 layout: [b, s*64+ci, h*112+w], s=0 -> row h, s=1 -> row h+1
    xd = np.zeros((B_FULL, 128, XCOLS), np.float16)
    flat = x16.reshape(B_FULL, C_IN, H * W)
    xd[:, :C_IN] = flat[:, :, :XCOLS]
    xd[:, C_IN:] = flat[:, :, W : W + XCOLS]
    # tile2 layout: halves are shifts (2,0) and (2,1) of the image
    xp = np.zeros((B_FULL, C_IN, H + 2, W + 1), np.float16)
    xp[:, :, :H, :W] = x16
    x2d = np.empty((B_FULL, 128, XCOLS), np.float16)
    x2d[:, :C_IN] = xp[:, :, 2 : 2 + 111, 0:W].reshape(B_FULL, C_IN, -1)[
        :, :, :XCOLS
    ]
    x2d[:, C_IN:] = xp[:, :, 2 : 2 + 111, 1 : 1 + W].reshape(B_FULL, C_IN, -1)[
        :, :, :XCOLS
    ]
    w5 = _prep_weights(weights)
    in_maps = [
        {
            "x": xd[B_CORE * i : B_CORE * (i + 1)],
            "x2": x2d[B_CORE * i : B_CORE * (i + 1)],
            "w": w5,
        }
        for i in range(N_CORES)
    ]
    res = run_bass_kernel_spmd(
        nc, in_maps, core_ids=list(range(N_CORES)), trace=_trace
    )
    out = np.concatenate(
        [res.results[i]["y"].astype(np.float32) for i in range(N_CORES)], axis=0
    )
    if _trace:
        return out, res
    return out


# revision 8
# speedup vs baseline: 1.1611x; 1.1611x over previous
"""Trainium2 Bass kernel: 3x3 VALID conv2d, stride 1.

Full input [32, 64, 112, 112] f32 + weights [128, 64, 3, 3] f32
-> output [32, 128, 110, 110] f32.

Data-parallel across 8 NeuronCores: 4 images per core.

Per-core formulation: conv as PE matmuls, out = lhsT.T @ rhs with
K (contraction, partitions) = 128 = two (ky,kx) taps x 64 channels,
M (out partitions) = 128 output channels,
N (moving free dim) = 4 input-width rows = 448 (<= 512, one PSUM bank).
The 2 rightmost columns of each 112-wide row are conv garbage; the
PSUM->SBUF copy compacts to the valid 110 columns.

Tap fusion (5 matmuls per chunk instead of the naive 9):
  tile1 = host-duplicated: partitions 0..63 = image rows h (shift (0,0)),
          partitions 64..127 = rows h+1 (shift (1,0)).
          m=0..2 fuse taps (0,kx)+(1,kx), rhs offset y0*W+kx.
  tile2 = host-duplicated second layout: partitions 0..63 = shift (2,0),
          partitions 64..127 = shift (2,1).
          m=3 fuses taps (2,0)+(2,1) at offset y0*W; m=4 is the lone
          (2,2) tap from half A at offset y0*W+2 (half-B weights zero).

Inputs are cast to fp16 on the host (full-rate PE streaming, fp32 PSUM
accumulation). Output is written fp16 (halves the store traffic) and
upcast to fp32 on the host; overall rel err ~5e-4.

Schedule: chunk-major in interleaved pairs (m0c0 m0c1 m1c0 .. m4c1) so
consecutive matmuls alternate PSUM banks and each chunk's PSUM drains
right after its stop matmul (weight reloads are per-matmul anyway and
hide under the 190ns stream). Copies split rows across vector+scalar.
All input DMAs ride the gpsimd SWDGE flood in consumption order
(weights first, tile1/tile2 bands interleaved, image 0 fine-grained);
outputs go on the SP HWDGE queue, one DMA per 4 chunks.
"""

import numpy as np

B_FULL = 32
N_CORES = 8
B_CORE = B_FULL // N_CORES  # 4 images per core
C_IN = 64
C_OUT = 128
H = W = 112
OH = OW = 110
XCOLS = 12322  # max col accessed: 108*112 + 2 + 224

_NC = None


def _build():
    from contextlib import ExitStack

    import concourse.tile as tile
    from concourse import bacc, mybir

    nc = bacc.Bacc("TRN2", target_bir_lowering=False, debug=False)
    x = nc.dram_tensor(
        "x", [B_CORE, 128, XCOLS], mybir.dt.float16, kind="ExternalInput"
    )
    x2 = nc.dram_tensor(
        "x2", [B_CORE, 128, XCOLS], mybir.dt.float16, kind="ExternalInput"
    )
    w = nc.dram_tensor("w", [128, 5, 128], mybir.dt.float16, kind="ExternalInput")
    y = nc.dram_tensor(
        "y", [B_CORE, C_OUT, OH, OW], mybir.dt.float16, kind="ExternalOutput"
    )

    chunks = []
    for b in range(B_CORE):
        y0 = 0
        for r in [4] * 27 + [2]:  # 27*4 + 2 = 110 output rows
            chunks.append((b, y0, r))
            y0 += r
    assert len(chunks) % 4 == 0

    # row bands, finer for image 0 (PE ramp); same grid for tile1/tile2
    BANDS0 = [0, 6, 16, 34, 61, 87, 111]
    BANDS = [0, 34, 61, 87, 111]

    with tile.TileContext(nc) as tc, ExitStack() as ctx:
        xpool = ctx.enter_context(tc.tile_pool(name="xp", bufs=4))
        x2pool = ctx.enter_context(tc.tile_pool(name="x2p", bufs=4))
        wpool = ctx.enter_context(tc.tile_pool(name="wp", bufs=1))
        opool = ctx.enter_context(tc.tile_pool(name="op", bufs=7))
        ppool = ctx.enter_context(tc.tile_pool(name="pp", bufs=8, space="PSUM"))

        wt = wpool.tile([128, 5, 128], mybir.dt.float16)
        nc.gpsimd.dma_start(wt[:], w.ap())

        xa = x.ap()
        x2a = x2.ap()
        ya = y.ap()

        xtiles = [None] * B_CORE
        x2tiles = [None] * B_CORE

        def issue_image_loads(b, t2_too=True):
            bands = BANDS0 if b == 0 else BANDS
            xt = xpool.tile([128, XCOLS], mybir.dt.float16, tag="xt")
            xtiles[b] = xt
            if t2_too:
                issue_x2_load(b)
            for lo, hi in zip(bands, bands[1:]):
                l, h2 = lo * W, min(hi * W, XCOLS)
                nc.gpsimd.dma_start(xt[:, l:h2], xa[b][:, l:h2])
                if t2_too:
                    nc.gpsimd.dma_start(
                        x2tiles[b][:, l:h2], x2a[b][:, l:h2]
                    )

        def issue_x2_load(b, bands=None):
            x2t = x2pool.tile([128, XCOLS], mybir.dt.float16, tag="x2t")
            x2tiles[b] = x2t
            if bands is not None:
                for lo, hi in zip(bands, bands[1:]):
                    l, h2 = lo * W, min(hi * W, XCOLS)
                    nc.gpsimd.dma_start(x2t[:, l:h2], x2a[b][:, l:h2])

        def issue_chunk_pair(c):
            """Two chunks, matmuls interleaved to alternate PSUM banks."""
            pair = chunks[c : c + 2]
            pts = []
            for b, y0, rows in pair:
                pts.append(
                    ppool.tile([128, 448], mybir.dt.float32, name="pt", tag="pt")
                )
            for m in range(5):
                for (b, y0, rows), pt in zip(pair, pts):
                    n = rows * W
                    if m < 3:
                        rhs = xtiles[b][:, y0 * W + m : y0 * W + m + n]
                    elif m == 3:
                        rhs = x2tiles[b][:, y0 * W : y0 * W + n]
                    else:
                        rhs = x2tiles[b][:, y0 * W + 2 : y0 * W + 2 + n]
                    nc.tensor.matmul(
                        pt[:, 0:n],
                        wt[:, m, :],
                        rhs,
                        start=(m == 0),
                        stop=(m == 4),
                        skip_group_check=True,
                    )
            return pair, pts

        def issue_quads(c0, c1):
            """Output tile + sync DMA per chunk pair (8 rows); per-chunk
            copies split rows across vector+scalar."""
            for c in range(c0, c1, 2):
                pair, pts = issue_chunk_pair(c)
                b0, y00, _ = pair[0]
                total_rows = sum(r for _, _, r in pair)
                assert all(b == b0 for b, _, _ in pair)
                ot = opool.tile([128, 8 * OW], mybir.dt.float16, tag="ot")
                off = 0
                for (b, y0, rows), pt in zip(pair, pts):
                    psrc = pt[:].rearrange("p (r c) -> p r c", c=W)
                    odst = ot[:, off : off + rows * OW].rearrange(
                        "p (r c) -> p r c", c=OW
                    )
                    rh = rows // 2
                    nc.vector.tensor_copy(
                        odst[:, 0:rh], psrc[:, 0:rh, 0:OW]
                    )
                    nc.scalar.copy(
                        odst[:, rh:rows], psrc[:, rh:rows, 0:OW]
                    )
                    off += rows * OW
                nc.sync.dma_start(
                    ya[b0].rearrange("c h w -> c (h w)")[
                        :, y00 * OW : y00 * OW + total_rows * OW
                    ],
                    ot[:, 0 : total_rows * OW],
                )

        # all loads issued up front in consumption order: with 4 x2
        # buffers there are no WAR waits, so the whole 25MB input flood
        # enters the (roughly FIFO) DMA queues ahead of the outputs and
        # each image's bands land before its compute window.
        for b in range(B_CORE):
            issue_image_loads(b)
        issue_quads(0, len(chunks))

    nc.compile()
    return nc


def _get_nc():
    global _NC
    if _NC is None:
        _NC = _build()
    return _NC


def _prep_weights(weights: np.ndarray) -> np.ndarray:
    # m=0..2: w5[ci, m, co] = w[co, ci, 0, m];  w5[64+ci, m, co] = w[co, ci, 1, m]
    # m=3:    w5[ci, 3, co] = w[co, ci, 2, 0];  w5[64+ci, 3, co] = w[co, ci, 2, 1]
    # m=4:    w5[ci, 4, co] = w[co, ci, 2, 2];  w5[64+ci, 4, co] = 0
    w = np.asarray(weights, dtype=np.float32)
    wt = w.transpose(1, 2, 3, 0)  # [ci, ky, kx, co]
    w5 = np.zeros((128, 5, 128), np.float32)
    w5[0:64, 0:3, :] = wt[:, 0, :, :]
    w5[64:128, 0:3, :] = wt[:, 1, :, :]
    w5[0:64, 3, :] = wt[:, 2, 0, :]
    w5[64:128, 3, :] = wt[:, 2, 1, :]
    w5[0:64, 4, :] = wt[:, 2, 2, :]
    return w5.astype(np.float16)


def kernel(input_image: np.ndarray, weights: np.ndarray, _trace: bool = False):
    from concourse.bass_utils import run_bass_kernel_spmd

    nc = _get_nc()
    x16 = np.asarray(input_image).astype(np.float16)  # [32, 64, 112, 112]
    # tile1 layout: [b, s*64+ci, h*112+w], s=0 -> row h, s=1 -> row h+1
    xd = np.zeros((B_FULL, 128, XCOLS), np.float16)
    flat = x16.reshape(B_FULL, C_IN, H * W)
    xd[:, :C_IN] = flat[:, :, :XCOLS]
    xd[:, C_IN:] = flat[:, :, W : W + XCOLS]
    # tile2 layout: halves are shifts (2,0) and (2,1) of the image
    xp = np.zeros((B_FULL, C_IN, H + 2, W + 1), np.float16)
    xp[:, :, :H, :W] = x16
    x2d = np.empty((B_FULL, 128, XCOLS), np.float16)
    x2d[:, :C_IN] = xp[:, :, 2 : 2 + 111, 0:W].reshape(B_FULL, C_IN, -1)[
        :, :, :XCOLS
    ]
    x2d[:, C_IN:] = xp[:, :, 2 : 2 + 111, 1 : 1 + W].reshape(B_FULL, C_IN, -1)[
        :, :, :XCOLS
    ]
    w5 = _prep_weights(weights)
    in_maps = [
        {
            "x": xd[B_CORE * i : B_CORE * (i + 1)],
            "x2": x2d[B_CORE * i : B_CORE * (i + 1)],
            "w": w5,
        }
        for i in range(N_CORES)
    ]
    res = run_bass_kernel_spmd(
        nc, in_maps, core_ids=list(range(N_CORES)), trace=_trace
    )
    out = np.concatenate(
        [res.results[i]["y"].astype(np.float32) for i in range(N_CORES)], axis=0
    )
    if _trace:
        return out, res
    return out
